# revision 1
# baseline (speedup 1.0000x reference)
"""Causal cross-attention kernel for 8 TRN2 NeuronCores.

Sharding: data-parallel over batch (B=2) x tensor-parallel over head
groups (16 heads -> 4 groups of 4). Core c handles batch c//4, heads
[4*(c%4), 4*(c%4)+4). Each core computes its partial output projection
(w_out rows for its heads); the host sums the 4 partials per batch
(the "all-reduce"), adds b_out, and fixes the fully-masked row 0.

Device dataflow per core (all matmuls in natural layouts, no device
transposes; activations host-transposed once):
  qT[f',s] = sum_f wq[f,f'] xT_from[f,s]     (lhsT=wq tile, rhs=xT tile)
  kT[f',s] = sum_f wk[f,f'] xT_to[f,s]
  v[z,f']  = sum_f xT_to[f,z] wv[f,f']       (lhsT=xT tile, rhs=wv)
  scoresT[z,s] = sum_d kT[d,z] qT[d,s]       (per head, K=64)
  P = exp(scoresT + causal_mask)             (no max-subtraction; logits
                                              are bounded ~|50| so exp is
                                              safe in f32, masked -> 0)
  out'T[d|1,s] = sum_z v'[z,d|1] P[z,s]      (v' has a ones column ->
                                              row 64 = softmax denom)
  attn_outT = out'T[0:64] * (1/out'T[64])    (PE-broadcast of recip row)
  out[s,fo] = sum_hd attn_outT[hd,s] wo[hd,fo]
"""

import numpy as np
import concourse.bass as bass
import concourse.mybir as mybir
import concourse.tile as tile
from concourse.bass_utils import run_bass_kernel_spmd

B, S, F, H = 2, 2048, 1024, 16
NCORES = 8
HG = 4          # head groups (tensor-parallel degree per batch)
HPC = H // HG   # heads per core = 4
D = F // H      # head dim = 64
CW = HPC * D    # per-core projection width = 256
MASK_VAL = 1.0e12
SC = 512        # s-chunk for projections / scores
NZC = S // 128  # 16 z-chunks

f32 = mybir.dt.float32
f32r = mybir.dt.float32r

# Walrus encodes at most 1 sync wait on most TRN2 instructions; Tile can
# attach several. Redistribute excess waits onto preceding same-engine NOPs.


def _split_excess_waits(nc):
    for fn in nc.m.functions:
        for bb in fn.blocks:
            insts = list(bb.instructions)
            out = []
            changed = False
            for inst in insts:
                si = inst.sync_info
                waits = list(si.on_wait) if si is not None else []
                if len(waits) > 1:
                    changed = True
                    inst.sync_info = mybir.SyncInfo(
                        on_update=list(si.on_update), on_wait=waits[-1:]
                    )
                    for idx, w in enumerate(waits[:-1]):
                        nop = mybir.InstNoOp(name=f"{inst.name}-wsplit{idx}")
                        nop.engine = inst.engine
                        nop.sync_info = mybir.SyncInfo(on_update=[], on_wait=[w])
                        out.append(nop)
                out.append(inst)
            if changed:
                bb.instructions = out


def _round_f32r(x):
    u = np.ascontiguousarray(x, dtype=np.float32).view(np.uint32)
    u = ((u.astype(np.uint64) + 0x1000) & 0xFFFFE000).astype(np.uint32)
    return u.view(np.float32)


def _build():
    nc = bass.Bass()
    xf_d = nc.declare_dram_parameter("xf", [F, S], f32r, isOutput=False)
    xt_d = nc.declare_dram_parameter("xt", [F, S], f32r, isOutput=False)
    wq_d = nc.declare_dram_parameter("wq", [F, CW], f32r, isOutput=False)
    wk_d = nc.declare_dram_parameter("wk", [F, CW], f32r, isOutput=False)
    wv_d = nc.declare_dram_parameter("wv", [F, CW], f32r, isOutput=False)
    wo_d = nc.declare_dram_parameter("wo", [CW, F], f32r, isOutput=False)
    bq_d = nc.declare_dram_parameter("bq", [CW, 1], f32, isOutput=False)
    bk_d = nc.declare_dram_parameter("bk", [CW, 1], f32, isOutput=False)
    bv_d = nc.declare_dram_parameter("bv", [1, CW], f32r, isOutput=False)
    msk_d = nc.declare_dram_parameter("msk", [128, 896], f32, isOutput=False)
    out_d = nc.declare_dram_parameter("out", [S, F], f32, isOutput=True)

    nsc = S // SC  # 4

    with tile.TileContext(nc) as tc:
        with (
            tc.tile_pool(name="const", bufs=1) as cpool,
            tc.tile_pool(name="xf", bufs=1) as xfpool,
            tc.tile_pool(name="xt", bufs=2) as xtpool,
            tc.tile_pool(name="work", bufs=2) as wpool,
            tc.tile_pool(name="pbuf", bufs=3) as ppool,
            tc.tile_pool(name="outst", bufs=2) as opool,
            tc.tile_pool(name="ps_gen", bufs=3, space="PSUM") as ps_gen,
            tc.tile_pool(name="ps_av", bufs=2, space="PSUM") as ps_av,
            tc.tile_pool(name="ps_b", bufs=2, space="PSUM") as ps_b,
        ):
            # ---- persistent tiles ----
            wq = cpool.tile([128, 8, CW], f32r)
            wk = cpool.tile([128, 8, CW], f32r)
            wv = cpool.tile([128, 8, CW], f32r)
            wo = cpool.tile([128, 2, F], f32r)
            bq = cpool.tile([128, 2, 1], f32)
            bk = cpool.tile([128, 2, 1], f32)
            bv = cpool.tile([1, CW], f32r)
            msk = cpool.tile([128, 896], f32)
            ones1 = cpool.tile([1, 128], f32r)
            ones_f = cpool.tile([128, 128], f32)
            qT = cpool.tile([128, 2, S], f32r)
            kT = cpool.tile([128, 2, S], f32r)
            v1 = cpool.tile([128, NZC, HPC, D + 1], f32r)
            aoT = cpool.tile([128, 2, S], f32r)

            for k in range(8):
                nc.sync.dma_start(
                    out=wq[:, k, :], in_=wq_d[128 * k : 128 * (k + 1), :]
                )
                nc.sync.dma_start(
                    out=wk[:, k, :], in_=wk_d[128 * k : 128 * (k + 1), :]
                )
                nc.sync.dma_start(
                    out=wv[:, k, :], in_=wv_d[128 * k : 128 * (k + 1), :]
                )
            nc.sync.dma_start(
                out=wo[:], in_=wo_d[:].rearrange("(m p) c -> p m c", p=128)
            )
            nc.sync.dma_start(
                out=bq[:], in_=bq_d[:].rearrange("(m p) c -> p m c", p=128)
            )
            nc.sync.dma_start(
                out=bk[:], in_=bk_d[:].rearrange("(m p) c -> p m c", p=128)
            )
            nc.sync.dma_start(out=bv[:], in_=bv_d[:])
            nc.sync.dma_start(out=msk[:], in_=msk_d[:])
            nc.vector.memset(ones_f[:], 1.0)
            nc.vector.tensor_copy(ones1[:], ones_f[0:1, :])
            # ones column of v' (col D of each [128, D+1] block)
            nc.vector.tensor_copy(
                v1[:, :, :, D : D + 1],
                ones_f[:, :64].rearrange("p (a b c) -> p a b c", a=NZC, b=HPC),
            )

            # ---- phase 1: projections ----
            for sc in range(nsc):
                s0 = sc * SC
                xf = xfpool.tile([128, 8, SC], f32r, tag="xf")
                xt = xtpool.tile([128, 8, SC], f32r, tag="xt")
                for k in range(8):
                    nc.sync.dma_start(
                        out=xf[:, k, :], in_=xf_d[128 * k : 128 * (k + 1), s0 : s0 + SC]
                    )
                    nc.sync.dma_start(
                        out=xt[:, k, :], in_=xt_d[128 * k : 128 * (k + 1), s0 : s0 + SC]
                    )
                for m in range(2):
                    pq = ps_gen.tile([128, SC], f32, tag="psg")
                    for k in range(8):
                        nc.tensor.matmul(
                            pq[:],
                            wq[:, k, m * 128 : (m + 1) * 128],
                            xf[:, k, :],
                            start=(k == 0),
                            stop=(k == 7),
                        )
                    nc.scalar.activation(
                        qT[:, m, s0 : s0 + SC],
                        pq[:],
                        mybir.ActivationFunctionType.Identity,
                        bias=bq[:, m, :],
                    )
                    pk = ps_gen.tile([128, SC], f32, tag="psg")
                    for k in range(8):
                        nc.tensor.matmul(
                            pk[:],
                            wk[:, k, m * 128 : (m + 1) * 128],
                            xt[:, k, :],
                            start=(k == 0),
                            stop=(k == 7),
                        )
                    nc.scalar.activation(
                        kT[:, m, s0 : s0 + SC],
                        pk[:],
                        mybir.ActivationFunctionType.Identity,
                        bias=bk[:, m, :],
                    )
                for zz in range(SC // 128):
                    zc = sc * (SC // 128) + zz
                    pv = ps_gen.tile([128, SC], f32, tag="psg")
                    for k in range(8):
                        nc.tensor.matmul(
                            pv[:, :CW],
                            xt[:, k, zz * 128 : (zz + 1) * 128],
                            wv[:, k, :],
                            start=(k == 0),
                            stop=False,
                        )
                    nc.tensor.matmul(
                        pv[:, :CW], ones1[:, :], bv[:], start=False, stop=True
                    )
                    for h in range(HPC):
                        nc.scalar.copy(
                            v1[:, zc, h, 0:D], pv[:, h * D : (h + 1) * D]
                        )

            # ---- phase 2: attention ----
            # The normalization tail (1-lane DVE reciprocal -> PE broadcast
            # matmul) is slow; emitted inline it head-of-line-blocks the
            # in-order PE queue for ~4us per group and re-throttles HAM.
            # Instead: start the reciprocal right after the group's last AV
            # matmul, but defer the PE broadcast + multiply until a few
            # score matmuls into the NEXT group (pav stays live, bufs=2).
            pending = []

            def _fin_b():
                for m_, po_, s0_, pav_, recip_r_ in pending:
                    pb = ps_b.tile([D, SC], f32, tag="pb")
                    nc.tensor.matmul(
                        pb[:], ones1[:, :D], recip_r_[:], start=True, stop=True
                    )
                    sb = wpool.tile([D, SC], f32, tag="sb")
                    nc.scalar.copy(sb[:], pb[:])
                    nc.vector.tensor_tensor(
                        out=aoT[po_ : po_ + D, m_, s0_ : s0_ + SC],
                        in0=pav_[0:D, :],
                        in1=sb[:],
                        op=mybir.AluOpType.mult,
                    )
                pending.clear()

            for h in range(HPC):
                m, po = divmod(h, 2)
                po *= 64
                for sc in range(nsc):
                    s0 = sc * SC
                    pav = ps_av.tile([D + 1, SC], f32, tag="pav")
                    nz = (sc + 1) * (SC // 128)
                    for zc in range(nz):
                        z0 = zc * 128
                        ps = ps_gen.tile([128, SC], f32, tag="psg")
                        nc.tensor.matmul(
                            ps[:],
                            kT[po : po + D, m, z0 : z0 + 128],
                            qT[po : po + D, m, s0 : s0 + SC],
                            start=True,
                            stop=True,
                        )
                        p = ppool.tile([128, SC], f32r, tag="p")
                        if z0 >= s0:
                            mstart = (s0 - z0) + 384
                            masked = wpool.tile([128, SC], f32, tag="masked")
                            nc.vector.tensor_tensor(
                                out=masked[:],
                                in0=ps[:],
                                in1=msk[:, mstart : mstart + SC],
                                op=mybir.AluOpType.add,
                            )
                            nc.scalar.activation(
                                p[:], masked[:], mybir.ActivationFunctionType.Exp
                            )
                        else:
                            nc.scalar.activation(
                                p[:], ps[:], mybir.ActivationFunctionType.Exp
                            )
                        nc.tensor.matmul(
                            pav[:],
                            v1[:, zc, h, :],
                            p[:],
                            start=(zc == 0),
                            stop=(zc == nz - 1),
                        )
                        if zc == 2:
                            _fin_b()
                    recip = wpool.tile([1, SC], f32, tag="recip")
                    nc.vector.reciprocal(recip[:], pav[D : D + 1, :])
                    recip_r = wpool.tile([1, SC], f32r, tag="recip_r")
                    nc.vector.tensor_scalar_min(recip_r[:], recip[:], 1.0e30)
                    pending.append((m, po, s0, pav, recip_r))
            _fin_b()

            # ---- phase 3: output projection ----
            for so in range(S // 128):
                s0 = so * 128
                ost = opool.tile([128, F], f32, tag="ost")
                for fo in range(2):
                    po_ = ps_gen.tile([128, SC], f32, tag="psg")
                    for m in range(2):
                        nc.tensor.matmul(
                            po_[:],
                            aoT[:, m, s0 : s0 + 128],
                            wo[:, m, fo * SC : (fo + 1) * SC],
                            start=(m == 0),
                            stop=(m == 1),
                        )
                    nc.scalar.copy(ost[:, fo * SC : (fo + 1) * SC], po_[:])
                nc.sync.dma_start(out=out_d[s0 : s0 + 128, :], in_=ost[:])

    _split_excess_waits(nc)
    return nc


_CACHE = {}


def _get_nc():
    if "nc" not in _CACHE:
        _CACHE["nc"] = _build()
    return _CACHE["nc"]


def _ensure_ntff_hook():
    """The agent image's antenv lacks axon_hooks, so run_bass_kernel_spmd's
    trace path can't import it. Synthesize the module and install the
    ctypes NTFF hook from trn_agent_boot (same thing boot() would do)."""
    import sys
    import types

    if "antenv.axon_hooks" not in sys.modules:
        mod = types.ModuleType("antenv.axon_hooks")
        holder = [None]
        mod.set_axon_ntff_profile_hook = lambda h: holder.__setitem__(0, h)
        mod.get_axon_ntff_profile_hook = lambda: holder[0]
        sys.modules["antenv.axon_hooks"] = mod
        import antenv

        antenv.axon_hooks = mod
    import antenv.axon_hooks as ah

    if ah.get_axon_ntff_profile_hook() is None:
        try:
            from trn_agent_boot.trn_boot import _ntff_profile_via_ctypes

            ah.set_axon_ntff_profile_hook(
                _ntff_profile_via_ctypes("/opt/axon/libaxon_pjrt.so")
            )
        except Exception:
            pass


def _host_mask():
    i = np.arange(128)[:, None]
    m = np.arange(896)[None, :]
    return np.where(i >= (m - 384), -np.float32(MASK_VAL), np.float32(0.0)).astype(
        np.float32
    )


def kernel(attend_from, attend_to, w_q, b_q, w_kv, b_kv, w_out, b_out, _trace=False):
    attend_from = np.asarray(attend_from, dtype=np.float32)
    attend_to = np.asarray(attend_to, dtype=np.float32)
    w_q = np.asarray(w_q, dtype=np.float32)
    b_q = np.asarray(b_q, dtype=np.float32)
    w_kv = np.asarray(w_kv, dtype=np.float32)
    b_kv = np.asarray(b_kv, dtype=np.float32)
    w_out = np.asarray(w_out, dtype=np.float32)
    b_out = np.asarray(b_out, dtype=np.float32)

    msk = _host_mask()
    xT = [_round_f32r(attend_from[b].T) for b in range(B)]
    xTt = [_round_f32r(attend_to[b].T) for b in range(B)]

    in_maps = []
    for c in range(NCORES):
        b, hg = divmod(c, HG)
        cols = slice(hg * CW, (hg + 1) * CW)
        in_maps.append(
            {
                "xf": xT[b],
                "xt": xTt[b],
                "wq": _round_f32r(w_q[:, cols]),
                "wk": _round_f32r(w_kv[:, cols]),
                "wv": _round_f32r(w_kv[:, F:][:, cols]),
                "wo": _round_f32r(w_out[cols, :]),
                "bq": np.ascontiguousarray(b_q[cols].reshape(CW, 1)),
                "bk": np.ascontiguousarray(b_kv[cols].reshape(CW, 1)),
                "bv": _round_f32r(b_kv[F:][cols].reshape(1, CW)),
                "msk": msk,
                "out": np.zeros((S, F), np.float32),
            }
        )

    nc = _get_nc()
    if _trace:
        _ensure_ntff_hook()
    res = run_bass_kernel_spmd(nc, in_maps, list(range(NCORES)), trace=_trace)

    out = np.zeros((B, S, F), np.float64)
    for c in range(NCORES):
        b = c // HG
        out[b] += res.results[c]["out"].astype(np.float64)
    out += b_out.astype(np.float64)[None, None, :]

    # Row 0 of the reference is fully masked -> softmax is exactly uniform
    # over all Z positions; compute it directly on the host.
    w_v = w_kv[:, F:].astype(np.float64)
    for b in range(B):
        val_mean = attend_to[b].astype(np.float64).mean(axis=0) @ w_v + b_kv[
            F:
        ].astype(np.float64)
        out[b, 0, :] = val_mean @ w_out.astype(np.float64) + b_out.astype(np.float64)

    if _trace:
        kernel._last_result = res
    return out.astype(np.float32)



# revision 33
# speedup vs baseline: 1.3531x; 1.3531x over previous
"""Causal cross-attention kernel for 8 TRN2 NeuronCores.

Sharding: data-parallel over batch (B=2) x tensor-parallel over head
groups (16 heads -> 4 groups of 4). Core c handles batch c//4, heads
[4*(c%4), 4*(c%4)+4). Each core computes its partial output projection
(w_out rows for its heads); the host sums the 4 partials per batch
(the "all-reduce"), adds b_out, and fixes the fully-masked row 0.

Precision: all matmuls run 16-bit operands (1 PE cycle/row — fp32/f32r
modes cost ~3x). The Q/K path is fp16 (11-bit mantissa; exp() amplifies
logit rounding, bf16 there costs ~1.8e-2 rel err vs ~5e-3 for fp16).
P=exp(scores) can reach e^45 so it must be bf16 (fp16 max 65504), which
forces v1 (the other AV operand) to bf16 too. PSUM accumulation is f32
throughout.

Schedule: sc-major (s-chunks of 512) so PE-heavy projection/output
phases interleave with the ACT-heavy exp phase:
  per sc: q/k proj -> flush prev norm -> v proj -> out-proj(sc-1)
          -> per head: score-pairs/exp/AV (1-ahead software pipeline)
Scores/exp/mask process z-block PAIRS ([128,1024] tiles spanning 2 PSUM
banks) to halve ACT/Pool per-instruction overheads. Masking adds a
precomputed [-1e12/0] tile on the Pool engine (in-place in PSUM).
Softmax denominators (row D of pav, via the ones-column of v1) are
clamped (Pool), inverted two-heads-at-a-time with the fast DVE
reciprocal approximation, rounded to bf16, PE-broadcast to 128
partitions with a block-one-hot lhsT, and multiplied into aoT on DVE.
"""

import numpy as np
import concourse.bass as bass
import concourse.mybir as mybir
import concourse.tile as tile
from concourse.bass_utils import run_bass_kernel_spmd

B, S, F, H = 2, 2048, 1024, 16
NCORES = 8
HG = 4          # head groups (tensor-parallel degree per batch)
HPC = H // HG   # heads per core = 4
D = F // H      # head dim = 64
CW = HPC * D    # per-core projection width = 256
MASK_VAL = 1.0e12
SC = 512        # s-chunk
NSC = S // SC   # 4

f32 = mybir.dt.float32
f16 = mybir.dt.float16
bf16 = mybir.dt.bfloat16

# Walrus encodes at most 1 sync wait on most TRN2 instructions; Tile can
# attach several. Redistribute excess waits onto preceding same-engine NOPs.


def _split_excess_waits(nc):
    for fn in nc.m.functions:
        for bb in fn.blocks:
            insts = list(bb.instructions)
            out = []
            changed = False
            for inst in insts:
                si = inst.sync_info
                waits = list(si.on_wait) if si is not None else []
                if len(waits) > 1:
                    changed = True
                    inst.sync_info = mybir.SyncInfo(
                        on_update=list(si.on_update), on_wait=waits[-1:]
                    )
                    for idx, w in enumerate(waits[:-1]):
                        nop = mybir.InstNoOp(name=f"{inst.name}-wsplit{idx}")
                        nop.engine = inst.engine
                        nop.sync_info = mybir.SyncInfo(on_update=[], on_wait=[w])
                        out.append(nop)
                out.append(inst)
            if changed:
                bb.instructions = out


def _build():
    nc = bass.Bass()
    xf_d = nc.declare_dram_parameter("xf", [F, S], f16, isOutput=False)
    xt_d = nc.declare_dram_parameter("xt", [F, S], f16, isOutput=False)
    wq_d = nc.declare_dram_parameter("wq", [F, CW], f16, isOutput=False)
    wk_d = nc.declare_dram_parameter("wk", [F, CW], f16, isOutput=False)
    wv_d = nc.declare_dram_parameter("wv", [F, CW], f16, isOutput=False)
    wo_d = nc.declare_dram_parameter("wo", [CW, F], f16, isOutput=False)
    bq_d = nc.declare_dram_parameter("bq", [CW, 1], f32, isOutput=False)
    bk_d = nc.declare_dram_parameter("bk", [CW, 1], f32, isOutput=False)
    bv_d = nc.declare_dram_parameter("bv", [1, CW], f16, isOutput=False)
    msk_d = nc.declare_dram_parameter("msk", [128, 128], f32, isOutput=False)
    out_d = nc.declare_dram_parameter("out", [S, F], f16, isOutput=True)

    with tile.TileContext(nc) as tc:
        with (
            tc.tile_pool(name="const", bufs=1) as cpool,
            tc.tile_pool(name="xf", bufs=2) as xfpool,
            tc.tile_pool(name="xt", bufs=2) as xtpool,
            tc.tile_pool(name="pbuf", bufs=3) as ppool,
            tc.tile_pool(name="work", bufs=2) as wpool,
            tc.tile_pool(name="outst", bufs=2) as opool,
            tc.tile_pool(name="ps_big", bufs=2, space="PSUM") as ps_big,
            tc.tile_pool(name="ps_av", bufs=3, space="PSUM") as ps_av,
            tc.tile_pool(name="ps_b", bufs=1, space="PSUM") as ps_b,
        ):
            # ---- persistent tiles ----
            wq = cpool.tile([128, 8, CW], f16)
            wk = cpool.tile([128, 8, CW], f16)
            wv = cpool.tile([128, 8, CW], f16)
            wo = cpool.tile([128, 2, F], f16)
            bq = cpool.tile([128, 2, 1], f32)
            bk = cpool.tile([128, 2, 1], f32)
            bv = cpool.tile([1, CW], f16)
            msk = cpool.tile([128, 128], f32)
            ones_v = cpool.tile([1, 128], f16)
            ones_b = cpool.tile([1, 128], bf16)
            nbias = cpool.tile([128, 1], f32)
            qT = cpool.tile([128, 2, S], f16)
            kT = cpool.tile([128, 2, S], f16)
            v1 = cpool.tile([128, S // 128, HPC, D + 1], bf16)
            aoT = cpool.tile([128, 2, S], f16)

            nc.sync.dma_start(
                out=wq[:], in_=wq_d[:].rearrange("(k p) c -> p k c", p=128)
            )
            nc.sync.dma_start(
                out=wk[:], in_=wk_d[:].rearrange("(k p) c -> p k c", p=128)
            )
            nc.sync.dma_start(
                out=wv[:], in_=wv_d[:].rearrange("(k p) c -> p k c", p=128)
            )
            nc.sync.dma_start(
                out=wo[:], in_=wo_d[:].rearrange("(m p) c -> p m c", p=128)
            )
            nc.sync.dma_start(
                out=bq[:], in_=bq_d[:].rearrange("(m p) c -> p m c", p=128)
            )
            nc.sync.dma_start(
                out=bk[:], in_=bk_d[:].rearrange("(m p) c -> p m c", p=128)
            )
            nc.sync.dma_start(out=bv[:], in_=bv_d[:])
            nc.sync.dma_start(out=msk[:], in_=msk_d[:])
            nc.vector.memset(ones_v[:], 1.0)
            nc.vector.memset(ones_b[:], 1.0)
            nc.vector.memset(nbias[:], -12.0)
            # ones column of v1 (col D) -> row D of pav = softmax denominator
            nc.vector.memset(v1[:, :, :, D : D + 1], 1.0)

            pending_norm = []

            def flush_norm():
                for m_, po_, sc_, pav_, rcb_ in pending_norm:
                    pb = ps_b.tile([D, SC], f32, tag="pb")
                    nc.tensor.matmul(
                        pb[:], ones_b[:, :D], rcb_[:, :], start=True, stop=True
                    )
                    sb = wpool.tile([D, SC], f32, tag="sb")
                    nc.vector.tensor_copy(sb[:], pb[:])
                    s0_ = sc_ * SC
                    nc.vector.tensor_tensor(
                        out=aoT[po_ : po_ + D, m_, s0_ : s0_ + SC],
                        in0=pav_[0:D, :],
                        in1=sb[:, :],
                        op=mybir.AluOpType.mult,
                    )
                pending_norm.clear()

            def emit_out_proj(sc):
                for so in range(SC // 128):
                    s0o = sc * SC + so * 128
                    pop = ps_big.tile([128, 2, SC], f32, tag="pair")
                    for fo in range(2):
                        for m in range(2):
                            nc.tensor.matmul(
                                pop[:, fo, :],
                                aoT[:, m, s0o : s0o + 128],
                                wo[:, m, fo * SC : (fo + 1) * SC],
                                start=(m == 0),
                                stop=(m == 1),
                            )
                    ost = opool.tile([128, F], f16, tag="ost")
                    nc.vector.tensor_copy(
                        ost[:].rearrange("p (a b) -> p a b", a=2), pop[:, :, :]
                    )
                    nc.sync.dma_start(out=out_d[s0o : s0o + 128, :], in_=ost[:])

            for sc in range(NSC):
                s0 = sc * SC
                xf = xfpool.tile([128, 8, SC], f16, tag="xf")
                xt = xtpool.tile([128, 8, SC], f16, tag="xt")
                for k in range(8):
                    nc.sync.dma_start(
                        out=xf[:, k, :],
                        in_=xf_d[128 * k : 128 * (k + 1), s0 : s0 + SC],
                    )
                    nc.sync.dma_start(
                        out=xt[:, k, :],
                        in_=xt_d[128 * k : 128 * (k + 1), s0 : s0 + SC],
                    )

                # ---- q/k projections for this s-chunk ----
                pq = ps_big.tile([128, 2, SC], f32, tag="pair")
                for m in range(2):
                    for k in range(8):
                        nc.tensor.matmul(
                            pq[:, m, :],
                            wq[:, k, m * 128 : (m + 1) * 128],
                            xf[:, k, :],
                            start=(k == 0),
                            stop=(k == 7),
                        )
                    nc.vector.tensor_scalar_add(
                        qT[:, m, s0 : s0 + SC], pq[:, m, :], bq[:, m, :]
                    )
                pk = ps_big.tile([128, 2, SC], f32, tag="pair")
                for m in range(2):
                    for k in range(8):
                        nc.tensor.matmul(
                            pk[:, m, :],
                            wk[:, k, m * 128 : (m + 1) * 128],
                            xt[:, k, :],
                            start=(k == 0),
                            stop=(k == 7),
                        )
                    nc.vector.tensor_scalar_add(
                        kT[:, m, s0 : s0 + SC], pk[:, m, :], bk[:, m, :]
                    )

                # normalization tail of the previous chunk's last head pair
                flush_norm()

                # ---- v projection (z-chunks of this s-chunk) ----
                for vp in range(2):
                    pv = ps_big.tile([128, 2, SC], f32, tag="pair")
                    for j in range(2):
                        zz = 2 * vp + j
                        for k in range(8):
                            nc.tensor.matmul(
                                pv[:, j, 0:CW],
                                xt[:, k, zz * 128 : (zz + 1) * 128],
                                wv[:, k, :],
                                start=(k == 0),
                                stop=False,
                            )
                        nc.tensor.matmul(
                            pv[:, j, 0:CW], ones_v[:, :], bv[:], start=False, stop=True
                        )
                    for j in range(2):
                        zc = sc * 4 + 2 * vp + j
                        nc.vector.tensor_copy(
                            v1[:, zc, :, 0:D],
                            pv[:, j, 0:CW].rearrange("p (h d) -> p h d", h=HPC),
                        )

                # previous chunk's output projection (PE-heavy, overlaps the
                # DVE/Pool normalization tail emitted above)
                if sc > 0:
                    emit_out_proj(sc - 1)

                # ---- attention for this s-chunk ----
                npair = 2 * (sc + 1)
                for h in range(HPC):
                    m, hh = divmod(h, 2)
                    po = hh * D
                    pav = ps_av.tile([D + 1, SC], f32, tag="pav")

                    # dz[p][j]: first valid s-column of z-block (2p+j); the
                    # scores/AV matmuls skip columns left of it (fully
                    # masked). Only a [128,128] staircase band at [dz,dz+128)
                    # mixes masked/unmasked.
                    def dz_of(p, j):
                        if p < 2 * sc:
                            return -1  # fully below the diagonal: no mask
                        return 256 * (p - 2 * sc) + 128 * j

                    def emit_av(p, pt):
                        for j in range(2):
                            d0 = max(0, dz_of(p, j))
                            nc.tensor.matmul(
                                pav[:, d0:SC],
                                v1[:, 2 * p + j, h, :],
                                pt[:, j, d0:SC],
                                start=(p == 0 and j == 0),
                                stop=(p == npair - 1 and j == 1),
                            )

                    prev = None
                    for p in range(npair):
                        psp = ps_big.tile([128, 2, SC], f32, tag="pair")
                        for j in range(2):
                            z0 = (2 * p + j) * 128
                            d0 = max(0, dz_of(p, j))
                            nc.tensor.matmul(
                                psp[:, j, d0:SC],
                                kT[po : po + D, m, z0 : z0 + 128],
                                qT[po : po + D, m, s0 + d0 : s0 + SC],
                                start=True,
                                stop=True,
                            )
                        for j in range(2):
                            d0 = dz_of(p, j)
                            if d0 >= 0:
                                nc.vector.tensor_tensor(
                                    out=psp[:, j, d0 : d0 + 128],
                                    in0=psp[:, j, d0 : d0 + 128],
                                    in1=msk[:, :],
                                    op=mybir.AluOpType.add,
                                )
                        # -12 shift: softmax-invariant, keeps den=sum(exp) in
                        # [2^-46, 2^58] — the ACT Ln table used for 1/den
                        # breaks beyond ~2^63 (logits reach ~51 -> den ~2^75).
                        pt = ppool.tile([128, 2, SC], bf16, tag="p")
                        nc.scalar.activation(
                            pt[:, :, :],
                            psp[:, :, :],
                            mybir.ActivationFunctionType.Exp,
                            bias=nbias[:, :],
                        )
                        if prev is not None:
                            emit_av(*prev)
                        prev = (p, pt)
                        if p == 1:
                            flush_norm()
                    emit_av(*prev)
                    # denominator -> 1/x as exp(-ln(x)) on ACT, bf16 out for
                    # the PE broadcast. den==0 (row 0) yields inf/NaN only in
                    # column s=0, which the host overwrites.
                    lg = wpool.tile([1, SC], f32, tag="lg")
                    nc.scalar.activation(
                        lg[:, :], pav[D : D + 1, :], mybir.ActivationFunctionType.Ln
                    )
                    rcb = wpool.tile([1, SC], bf16, tag="rcb")
                    nc.scalar.activation(
                        rcb[:, :],
                        lg[:, :],
                        mybir.ActivationFunctionType.Exp,
                        scale=-1.0,
                    )
                    pending_norm.append((m, po, sc, pav, rcb))

            flush_norm()
            emit_out_proj(NSC - 1)

    _split_excess_waits(nc)
    return nc


_CACHE = {}


def _get_nc():
    if "nc" not in _CACHE:
        _CACHE["nc"] = _build()
    return _CACHE["nc"]


def _ensure_ntff_hook():
    """The agent image's antenv lacks axon_hooks, so run_bass_kernel_spmd's
    trace path can't import it. Synthesize the module and install the
    ctypes NTFF hook from trn_agent_boot (same thing boot() would do)."""
    import sys
    import types

    if "antenv.axon_hooks" not in sys.modules:
        mod = types.ModuleType("antenv.axon_hooks")
        holder = [None]
        mod.set_axon_ntff_profile_hook = lambda h: holder.__setitem__(0, h)
        mod.get_axon_ntff_profile_hook = lambda: holder[0]
        sys.modules["antenv.axon_hooks"] = mod
        import antenv

        antenv.axon_hooks = mod
    import antenv.axon_hooks as ah

    if ah.get_axon_ntff_profile_hook() is None:
        try:
            from trn_agent_boot.trn_boot import _ntff_profile_via_ctypes

            ah.set_axon_ntff_profile_hook(
                _ntff_profile_via_ctypes("/opt/axon/libaxon_pjrt.so")
            )
        except Exception:
            pass


def _host_mask():
    # The causal band of every diagonal score block (z0 = s0 + dz) reduced to
    # its mixed [dz, dz+128) columns is the same staircase: element
    # (z = z0 + i, s = s0 + dz + c) is masked iff s <= z iff c <= i.
    i = np.arange(128)[:, None]
    c = np.arange(128)[None, :]
    return np.where(c <= i, -np.float32(MASK_VAL), np.float32(0.0)).astype(np.float32)


def kernel(attend_from, attend_to, w_q, b_q, w_kv, b_kv, w_out, b_out, _trace=False):
    attend_from = np.asarray(attend_from, dtype=np.float32)
    attend_to = np.asarray(attend_to, dtype=np.float32)
    w_q = np.asarray(w_q, dtype=np.float32)
    b_q = np.asarray(b_q, dtype=np.float32)
    w_kv = np.asarray(w_kv, dtype=np.float32)
    b_kv = np.asarray(b_kv, dtype=np.float32)
    w_out = np.asarray(w_out, dtype=np.float32)
    b_out = np.asarray(b_out, dtype=np.float32)

    msk = _host_mask()
    xT = [attend_from[b].T.astype(np.float16) for b in range(B)]
    xTt = [attend_to[b].T.astype(np.float16) for b in range(B)]

    in_maps = []
    for c in range(NCORES):
        b, hg = divmod(c, HG)
        cols = slice(hg * CW, (hg + 1) * CW)
        in_maps.append(
            {
                "xf": xT[b],
                "xt": xTt[b],
                "wq": w_q[:, cols].astype(np.float16),
                "wk": w_kv[:, cols].astype(np.float16),
                "wv": w_kv[:, F:][:, cols].astype(np.float16),
                "wo": w_out[cols, :].astype(np.float16),
                "bq": np.ascontiguousarray(b_q[cols].reshape(CW, 1)),
                "bk": np.ascontiguousarray(b_kv[cols].reshape(CW, 1)),
                "bv": b_kv[F:][cols].reshape(1, CW).astype(np.float16),
                "msk": msk,
                "out": np.zeros((S, F), np.float16),
            }
        )

    nc = _get_nc()
    if _trace:
        _ensure_ntff_hook()
    res = run_bass_kernel_spmd(nc, in_maps, list(range(NCORES)), trace=_trace)

    out = np.zeros((B, S, F), np.float64)
    for c in range(NCORES):
        b = c // HG
        out[b] += res.results[c]["out"].astype(np.float64)
    out += b_out.astype(np.float64)[None, None, :]

    # Row 0 of the reference is fully masked -> softmax is exactly uniform
    # over all Z positions (the -1e12 shift absorbs the logits in f32);
    # compute it directly on the host.
    w_v = w_kv[:, F:].astype(np.float64)
    for b in range(B):
        val_mean = attend_to[b].astype(np.float64).mean(axis=0) @ w_v + b_kv[
            F:
        ].astype(np.float64)
        out[b, 0, :] = val_mean @ w_out.astype(np.float64) + b_out.astype(np.float64)

    if _trace:
        kernel._last_result = res
    return out.astype(np.float32)
